# revision 41
# baseline (speedup 1.0000x reference)
"""Causal self-attention on 8 TRN2 NeuronCores.

Problem (hardcoded): B=4, T=2048, C=1024, H=16 heads, D=64.
  qkv = x @ W_in + b_in ; causal softmax attention ; out = y @ W_out + b_out

Sharding: core c handles batch b = c//2 and head-group g = c%2 (8 heads).
Each core computes its partial out-projection (sum over its heads' columns);
the host adds the two partials per batch plus b_out. No device collectives.

Device design:
  - All matmul operands are float32r (full-rate PE, ~1e-4 rel err).
  - x is pre-transposed on host; q pre-scaled by 1/sqrt(D) (folded into W_q).
  - Scores computed transposed: S^T[k, q] = k . q, so exp(S^T) = P^T feeds
    the PV matmul directly -- no on-chip transposes anywhere.
  - exp without max-subtraction (scores are ~N(0,1); fp32 exp is safe).
  - v carries a ones-column per head: PV matmul emits softmax denominators
    as its last output row for free.
  - Causal pipeline: projection of query-window w+1 and the out-projection
    of window w-1 are emitted as PE "filler" units interleaved into the
    ACT(exp)-bound attention stream of window w, keeping PE dense.
"""

import sys

for _p in ("/opt/trn_rl_repo", "/root/.axon_site/_ro/trn_rl_repo"):
    if _p not in sys.path:
        sys.path.append(_p)

import numpy as np

B, T, C = 4, 2048, 1024
H = 16  # total heads
HL = 8  # heads per core
D = 64  # head dim
P = 128
KO = C // P  # 8 contraction chunks
TQ = 512  # query-window width
NTQ = T // TQ  # 4 windows
G2 = 2  # key chunks per exp group

_CACHE = {}


def _build():
    import concourse.mybir as mybir
    import concourse.tile as tile
    from concourse import bacc

    fr = mybir.dt.float32r
    f32 = mybir.dt.float32

    nc = bacc.Bacc("TRN2", target_bir_lowering=False, debug=False, num_devices=8)

    xT = nc.dram_tensor("xT", [C, T], fr, kind="ExternalInput")
    w_qk = nc.dram_tensor("w_qk", [C, 2 * HL * D], fr, kind="ExternalInput")
    b_qk = nc.dram_tensor("b_qk", [2 * HL * D], f32, kind="ExternalInput")
    w_v = nc.dram_tensor("w_v", [C, HL * D], fr, kind="ExternalInput")
    b_v = nc.dram_tensor("b_v", [HL * D], fr, kind="ExternalInput")
    w_out = nc.dram_tensor("w_out", [HL * D, C], fr, kind="ExternalInput")
    masks = nc.dram_tensor("masks", [P, 896], fr, kind="ExternalInput")
    vones = nc.dram_tensor("vones", [P, 4 * HL], fr, kind="ExternalInput")
    out = nc.dram_tensor("out", [T, C], f32, kind="ExternalOutput")

    FQK = 2 * HL * D  # 1024 (q block then k block)
    FV = HL * D  # 512

    with tile.TileContext(nc) as tc:
        import contextlib
        from collections import deque

        ctx = contextlib.ExitStack()
        with ctx:
            persist = ctx.enter_context(tc.tile_pool(name="persist", bufs=1))
            qT_pool = ctx.enter_context(tc.tile_pool(name="qT", bufs=2))
            xT_pool = ctx.enter_context(tc.tile_pool(name="xT", bufs=1))
            pT_pool = ctx.enter_context(tc.tile_pool(name="pT", bufs=3))
            sm = ctx.enter_context(tc.tile_pool(name="sm", bufs=2))
            yT_pool = ctx.enter_context(tc.tile_pool(name="yT", bufs=2))
            o_pool = ctx.enter_context(tc.tile_pool(name="o", bufs=2))

            # ---- weights + first x window, in first-use order ----
            wqk_t = persist.tile([P, KO, FQK], fr)
            xT0_tiles = []
            for ko in range(KO):
                nc.sync.dma_start(wqk_t[:, ko], w_qk[ko * P : (ko + 1) * P, :])
                t_ = xT_pool.tile([P, TQ], fr, tag=f"xT{ko}", name=f"xT0_{ko}")
                nc.gpsimd.dma_start(t_, xT[ko * P : (ko + 1) * P, 0:TQ])
                xT0_tiles.append(t_)
            b_qk_sb = persist.tile([P, KO], f32)
            nc.sync.dma_start(b_qk_sb, b_qk.rearrange("(fo p) -> p fo", p=P))
            wv_t = persist.tile([P, KO, FV], fr)
            for ko in range(KO):
                nc.sync.dma_start(wv_t[:, ko], w_v[ko * P : (ko + 1) * P, :])
            bv_bc = persist.tile([P, FV], fr)
            nc.sync.dma_start(bv_bc, b_v[None, :].to_broadcast((P, FV)))
            mask_sb = persist.tile([P, 896], fr)
            nc.sync.dma_start(mask_sb, masks[:])
            w_out_sb = persist.tile([P, 4, C], fr)  # [p, do, n]
            for do in range(4):
                nc.sync.dma_start(
                    w_out_sb[:, do], w_out[do * P : (do + 1) * P, :]
                )

            # per-window persistent activations
            kT_w = []  # [p, kfo(4), TQ] per window
            v65_w = []  # [p, t4(4), HL, 65] per window
            for w in range(NTQ):
                kT_w.append(persist.tile([P, 4, TQ], fr, tag=f"kT{w}", name=f"kT{w}"))
                v65_w.append(persist.tile([P, 4, HL, D + 1], fr, tag=f"v65{w}", name=f"v65{w}"))
                nc.sync.dma_start(
                    v65_w[w][:, :, :, D],
                    vones.rearrange("p (n h) -> p n h", n=4),
                )

            # ---------------- unit builders ----------------
            def load_xT(w):
                tiles = []
                for ko in range(KO):
                    t_ = xT_pool.tile([P, TQ], fr, tag=f"xT{ko}")
                    nc.sync.dma_start(
                        t_, xT[ko * P : (ko + 1) * P, w * TQ : (w + 1) * TQ]
                    )
                    tiles.append(t_)
                return tiles

            def proj_qk_unit(w, fo, qT_next, xTs):
                def emit():
                    ps = ps_pj.tile([P, TQ], f32, tag="pj")
                    for ko in range(KO):
                        nc.tensor.matmul(
                            ps,
                            wqk_t[:, ko, fo * P : (fo + 1) * P],
                            xTs[ko],
                            start=(ko == 0),
                            stop=(ko == KO - 1),
                        )
                    dst = (
                        qT_next[:, fo] if fo < 4 else kT_w[w][:, fo - 4]
                    )
                    nc.vector.tensor_scalar(
                        dst,
                        ps,
                        b_qk_sb[:, fo : fo + 1],
                        None,
                        mybir.AluOpType.add,
                    )

                return emit

            def proj_v_unit(w, t4, xTs):
                def emit():
                    ps = ps_pj.tile([P, FV], f32, tag="pj")
                    for ko in range(KO):
                        nc.tensor.matmul(
                            ps,
                            xTs[ko][:, t4 * P : (t4 + 1) * P],
                            wv_t[:, ko],
                            start=(ko == 0),
                            stop=(ko == KO - 1),
                        )
                    nc.vector.tensor_tensor(
                        v65_w[w][:, t4, :, :D],
                        ps.rearrange("p (h d) -> p h d", h=HL),
                        bv_bc.rearrange("p (h d) -> p h d", h=HL),
                        mybir.AluOpType.add,
                    )

                return emit

            def op_unit(tq, ts_, yT_win, scalar_copy=False, tail_psum=False):
                def emit():
                    t0 = tq * TQ + ts_ * P
                    for n in range(2):
                        if tail_psum:
                            ps = ps_s.tile([P, 512], f32, tag="ps_s", name="ps_o")
                        else:
                            ps = ps_pj.tile([P, 512], f32, tag="pj")
                        for do in range(4):
                            nc.tensor.matmul(
                                ps,
                                yT_win[:, do, ts_ * P : (ts_ + 1) * P],
                                w_out_sb[:, do, n * 512 : (n + 1) * 512],
                                start=(do == 0),
                                stop=(do == 3),
                            )
                        o_sb = o_pool.tile([P, 512], f32, tag="o")
                        if scalar_copy:
                            nc.scalar.copy(o_sb, ps)
                        else:
                            nc.vector.tensor_copy(o_sb, ps)
                        nc.sync.dma_start(
                            out[t0 : t0 + P, n * 512 : (n + 1) * 512], o_sb
                        )

                return emit

            # paced filler drain
            class Pacer:
                def __init__(self, fillers, total_slots, backload=1.0, reserve=0):
                    self.fillers = deque(fillers)
                    self.total = max(1, total_slots)
                    self.n = len(fillers)
                    self.slot = 0
                    self.done = 0
                    self.backload = backload
                    self.reserve = reserve

                def tick(self):
                    self.slot += 1
                    want = min(
                        int(self.n * (self.slot / self.total) ** self.backload),
                        self.n - self.reserve,
                    )
                    while self.done < want and self.fillers:
                        self.fillers.popleft()()
                        self.done += 1

                def drain(self):
                    while self.fillers:
                        self.fillers.popleft()()

            def att_pair(tq, j, qT_cur, yT_win, pacer):
                """Heads 2j (partitions 0:64) and 2j+1 (64:128) packed:
                their K=64 S-matmuls run in disjoint PE row groups, one exp
                covers both heads' key-chunk."""
                nchunks = 4 * (tq + 1)
                hA, hB = 2 * j, 2 * j + 1
                qA = qT_cur[0:D, j, :]
                qB = qT_cur[D:P, j, :]
                psyA = ps_y.tile([D + 1, TQ], f32, tag="ps_yA", name="psyA")
                psyB = ps_y.tile([D + 1, TQ], f32, tag="ps_yB", name="psyB")
                prev = None
                for i in range(nchunks):
                    i4 = i - 4 * tq
                    diag = 0 <= i4 < 4
                    # skip fully-masked leading query columns of diagonal
                    # chunks; keep the moving dim >=256 for full-rate fp32r
                    col0 = min(P * i4, 256) if diag else 0
                    w = TQ - col0
                    pss = ps_s.tile([P, 2 * TQ], f32, tag="ps_s")
                    kslice = slice((i % 4) * P, (i % 4 + 1) * P)
                    nc.tensor.matmul(
                        pss[:, col0:TQ],
                        kT_w[i // 4][0:D, j, kslice],
                        qA[:, col0:TQ],
                        start=True,
                        stop=True,
                    )
                    nc.tensor.matmul(
                        pss[:, TQ + col0 : 2 * TQ],
                        kT_w[i // 4][D:P, j, kslice],
                        qB[:, col0:TQ],
                        start=True,
                        stop=True,
                    )
                    pT = pT_pool.tile([P, 2 * TQ], fr, tag="pT")
                    pss3 = pss.rearrange("p (c t) -> p c t", c=2)
                    pT3 = pT.rearrange("p (c t) -> p c t", c=2)
                    nc.scalar.activation(
                        pT3[:, :, col0:TQ],
                        pss3[:, :, col0:TQ],
                        mybir.ActivationFunctionType.Exp,
                    )
                    if diag:
                        off = 384 - 128 * i4
                        nc.vector.tensor_tensor(
                            pT3[:, :, col0:TQ],
                            pT3[:, :, col0:TQ],
                            mask_sb[:, off + col0 : off + TQ]
                            .unsqueeze(1)
                            .to_broadcast((P, 2, w)),
                            mybir.AluOpType.mult,
                        )
                    if prev is not None:
                        _pv(prev, psyA, psyB, nchunks, hA, hB)
                    prev = (pT, i, col0)
                    pacer.tick()
                _pv(prev, psyA, psyB, nchunks, hA, hB)
                # interleave A/B normalize chains so the GpSimd broadcast
                # round-trips overlap DVE work instead of stalling it
                stage = []
                for psy, pb in ((psyA, 0), (psyB, D)):
                    psy_sb = sm.tile([D + 1, TQ], fr, tag="psy_sb", name="psy_sb")
                    nc.vector.tensor_copy(psy_sb, psy)
                    rec_bc = sm.tile([D, TQ], fr, tag="rec_bc")
                    with nc.allow_low_precision(reason="fp32r is plenty"):
                        nc.vector.reciprocal(rec_bc[0:1, :], psy_sb[D : D + 1, :])
                    stage.append((psy_sb, rec_bc, pb))
                for psy_sb, rec_bc, pb in stage:
                    nc.gpsimd.partition_broadcast(rec_bc, rec_bc[0:1, :], channels=D)
                for psy_sb, rec_bc, pb in stage:
                    nc.vector.tensor_tensor(
                        yT_win[pb : pb + D, j, :],
                        psy_sb[:D, :],
                        rec_bc,
                        mybir.AluOpType.mult,
                    )

            def _pv(prev, psyA, psyB, nchunks, hA, hB):
                pT, i, col0 = prev
                nc.tensor.matmul(
                    psyA[:, col0:TQ],
                    v65_w[i // 4][:, i % 4, hA],
                    pT[:, col0:TQ],
                    start=(i == 0),
                    stop=(i == nchunks - 1),
                )
                nc.tensor.matmul(
                    psyB[:, col0:TQ],
                    v65_w[i // 4][:, i % 4, hB],
                    pT[:, TQ + col0 : 2 * TQ],
                    start=(i == 0),
                    stop=(i == nchunks - 1),
                )

            # ---------------- emission ----------------
            # window-0 projection: ko-outer so PE starts on the first chunks
            qT_cur = qT_pool.tile([P, 4, TQ], tag="qT", dtype=fr)
            with tc.tile_pool(name="pj0", bufs=1, space="PSUM") as pj0:
                ps_fo = [
                    pj0.tile([P, TQ], f32, tag=f"pj0_{fo}", name=f"pj0_{fo}")
                    for fo in range(KO)
                ]
                for ko in range(KO):
                    for fo in range(KO):
                        nc.tensor.matmul(
                            ps_fo[fo],
                            wqk_t[:, ko, fo * P : (fo + 1) * P],
                            xT0_tiles[ko],
                            start=(ko == 0),
                            stop=(ko == KO - 1),
                        )
                for fo in range(KO):
                    dst = qT_cur[:, fo] if fo < 4 else kT_w[0][:, fo - 4]
                    nc.vector.tensor_scalar(
                        dst,
                        ps_fo[fo],
                        b_qk_sb[:, fo : fo + 1],
                        None,
                        mybir.AluOpType.add,
                    )
                for t4 in range(4):
                    psv = pj0.tile([P, FV], f32, tag=f"pj0_{t4}", name=f"pj0v_{t4}")
                    for ko in range(KO):
                        nc.tensor.matmul(
                            psv,
                            xT0_tiles[ko][:, t4 * P : (t4 + 1) * P],
                            wv_t[:, ko],
                            start=(ko == 0),
                            stop=(ko == KO - 1),
                        )
                    nc.vector.tensor_tensor(
                        v65_w[0][:, t4, :, :D],
                        psv.rearrange("p (h d) -> p h d", h=HL),
                        bv_bc.rearrange("p (h d) -> p h d", h=HL),
                        mybir.AluOpType.add,
                    )
            ps_pj = ctx.enter_context(tc.tile_pool(name="ps_pj", bufs=2, space="PSUM"))
            ps_s = ctx.enter_context(tc.tile_pool(name="ps_s", bufs=2, space="PSUM"))
            ps_y = ctx.enter_context(tc.tile_pool(name="ps_y", bufs=1, space="PSUM"))

            yT_prev = None
            for tq in range(NTQ):
                fillers = []
                qT_next = None
                if tq + 1 < NTQ:
                    xTs = load_xT(tq + 1)
                    qT_next = qT_pool.tile([P, 4, TQ], tag="qT", dtype=fr)
                    for fo in range(KO):
                        fillers.append(proj_qk_unit(tq + 1, fo, qT_next, xTs))
                    for t4 in range(4):
                        fillers.append(proj_v_unit(tq + 1, t4, xTs))
                if yT_prev is not None:
                    for ts_ in range(4):
                        fillers.append(op_unit(tq - 1, ts_, yT_prev))
                yT_win = yT_pool.tile([P, 4, TQ], tag="yT", dtype=fr, name="yT_win")
                pacer = Pacer(
                    fillers,
                    total_slots=(HL // 2) * 4 * (tq + 1),
                    backload=3.0 if tq == NTQ - 1 else 1.0,
                    reserve=0,
                )
                if tq == 0 and fillers:
                    pacer.fillers.popleft()()
                    pacer.done += 1
                for j in range(HL // 2):
                    att_pair(tq, j, qT_cur, yT_win, pacer)
                pacer.drain()
                qT_cur = qT_next
                yT_prev = yT_win
            for ts_ in range(4):
                op_unit(NTQ - 1, ts_, yT_prev, scalar_copy=True, tail_psum=True)()

    nc.compile()
    return nc


def _get_nc():
    if "nc" not in _CACHE:
        _CACHE["nc"] = _build()
    return _CACHE["nc"]


def kernel(x, W_in, b_in, W_out, b_out):
    from concourse.bass_utils import run_bass_kernel_spmd

    x = np.asarray(x, dtype=np.float32)
    W_in = np.asarray(W_in, dtype=np.float32)
    b_in = np.asarray(b_in, dtype=np.float32)
    W_out = np.asarray(W_out, dtype=np.float32)
    b_out = np.asarray(b_out, dtype=np.float32)

    scale = 1.0 / np.sqrt(D)

    # causal mask master: M[p, u] = 1 if u >= p + 384
    u = np.arange(896)[None, :]
    p = np.arange(P)[:, None]
    mask = (u >= p + 384).astype(np.float32)
    vones_np = np.ones((P, 4 * HL), np.float32)

    in_maps = []
    for c in range(8):
        b, g = c // 2, c % 2
        qc = slice(g * HL * D, (g + 1) * HL * D)
        kc = slice(C + g * HL * D, C + (g + 1) * HL * D)
        vc = slice(2 * C + g * HL * D, 2 * C + (g + 1) * HL * D)
        w_qk = np.concatenate([W_in[:, qc] * scale, W_in[:, kc]], axis=1)
        b_qk = np.concatenate([b_in[qc] * scale, b_in[kc]])
        in_maps.append(
            {
                "xT": np.ascontiguousarray(x[b].T),
                "w_qk": np.ascontiguousarray(w_qk),
                "b_qk": np.ascontiguousarray(b_qk),
                "w_v": np.ascontiguousarray(W_in[:, vc]),
                "b_v": np.ascontiguousarray(b_in[vc]),
                "w_out": np.ascontiguousarray(W_out[g * HL * D : (g + 1) * HL * D, :]),
                "masks": mask,
                "vones": vones_np,
            }
        )

    global _last_in_maps
    _last_in_maps = in_maps
    nc = _get_nc()
    res = run_bass_kernel_spmd(nc, in_maps, list(range(8)))

    out = np.empty((B, T, C), np.float32)
    for b in range(B):
        out[b] = res.results[2 * b]["out"] + res.results[2 * b + 1]["out"] + b_out
    return out


if __name__ == "__main__":
    rng = np.random.default_rng(0)
    x = rng.standard_normal((B, T, C), dtype=np.float32)
    W_in = rng.standard_normal((C, 3 * C), dtype=np.float32) / np.sqrt(C)
    b_in = np.zeros(3 * C, np.float32)
    W_out = rng.standard_normal((C, C), dtype=np.float32) / np.sqrt(C)
    b_out = np.zeros(C, np.float32)
    y = kernel(x=x, W_in=W_in, b_in=b_in, W_out=W_out, b_out=b_out)
    print("ok", y.shape, y.dtype)
